# revision 22
# baseline (speedup 1.0000x reference)
"""Trainium2 Bass kernel for the branched cross-attention processor.

Problem (full shapes):
  hidden_states [4, 4096, 1280], encoder_hidden_states [4, 77, 2048],
  id_embedding [2, 32, 2048], Wq/Wout [1280,1280], Wk/Wv/Wid_k/Wid_v
  [2048,1280], bout [1280].  20 heads, dh=64.  Output [4, 4096, 1280].

Sharding: data-parallel over (batch, seq-half): core c handles batch c//2,
query rows (c%2)*2048 : (c%2+1)*2048.  K/V (109 keys) are computed
per-core for its batch.  All queries are independent (full cross
attention), so no collectives are needed.

Per-core pipeline (all matmuls float32r, N=512 => full PE rate):
  qT[j,sq]    = sum_i Wq[i,j] * hsT[i,sq]                (q projection, transposed)
  k~/v~       = [ehs @ Wk|Wv ; id @ Wid_k|Wid_v]          (109 keys, padded to 128
                rows: [0:77]=ehs, [77:96]=zero gap, [96:128]=id)
  kT          = transpose(k~)  (PE transpose)
  scoresT     = kT_h^T @ qT_h        [128keys, 512sq]  per (head, sq-chunk)
  probsT      = exp(0.125*scoresT + gapbias)   (gap rows get -1e30 -> exp 0)
  attnT_pair  = v_pair^T @ probsT    [128, 512]  (head h in rows 64*(h%2):+64)
  denom_rep   = ones128^T @ probsT   [128, 512]  (denominator replicated on all
                partitions by an all-ones stationary matrix -> no broadcast op)
  attnT_h     = attnT_pair[rows] * reciprocal(denom_rep)[rows]
  out         = attnT^T @ Wout + bout

fp32r ISA restriction: matmul dst must cover all 128 partitions (col_grp
0xf), so every matmul here has M=128; partial results use only the rows
that are valid.
"""

import os
import sys
import types

import numpy as np

# ---------------------------------------------------------------------------
# problem constants (hardcoded; kernel.py must be self-contained)
# ---------------------------------------------------------------------------
B = 4
S = 4096
H = 1280
C = 2048
TE = 77          # encoder tokens
TI = 32          # id tokens
HEADS = 20
DH = 64          # head dim
P = 128
L = 109          # TE + TI
LP = 128         # padded key count: [0:77]=ehs, [77:96]=gap, [96:128]=id
GAP0, GAP1 = TE, P - TI   # 77, 96
SC = 2048        # seq rows per core
NJ = H // P      # 10
NI = C // P      # 16
NCH = SC // 512  # 4 sq-chunks of 512
NT = SC // P     # 16 sq-tiles of 128
SCALE = 1.0 / 8.0
NCORES = 8
MCHUNKS = [(0, 512), (512, 512), (1024, 256)]

_NC_CACHE = {}


def _ensure_axon_hooks():
    """The image's antenv lacks axon_hooks; synthesize it so NTFF profiling
    (trace=True) works when test.py asks for it.  Harmless if unused."""
    if "antenv.axon_hooks" in sys.modules:
        return
    try:
        import antenv
        from trn_agent_boot.trn_boot import _ntff_profile_via_ctypes

        hook = _ntff_profile_via_ctypes("/opt/axon/libaxon_pjrt.so")
        m = types.ModuleType("antenv.axon_hooks")
        m.get_axon_ntff_profile_hook = lambda: hook
        m.set_axon_ntff_profile_hook = lambda h: None
        sys.modules["antenv.axon_hooks"] = m
        antenv.axon_hooks = m
    except Exception:
        pass


def build_nc():
    """Build + compile the per-core Bass program (SPMD: same NEFF, 8 cores)."""
    if "nc" in _NC_CACHE:
        return _NC_CACHE["nc"]

    import concourse.bass as bass
    import concourse.tile as tile
    from concourse import bacc, mybir
    from concourse.bass import ts

    F32 = mybir.dt.float32
    R = mybir.dt.float16      # matmul operand dtype (1 cyc/row, 10-bit mantissa)
    EXP = mybir.ActivationFunctionType.Exp

    nc = bacc.Bacc("TRN2", target_bir_lowering=False, debug=False, num_devices=NCORES)

    hsT = nc.dram_tensor("hsT", [H, SC], R, kind="ExternalInput").ap()
    xkvTp = nc.dram_tensor("xkvTp", [NI, P, LP], R, kind="ExternalInput").ap()
    wqp = nc.dram_tensor("wqp", [NJ, NJ, P, P], R, kind="ExternalInput").ap()
    # per-core HALF of the k/v weights (core parity picks the column half);
    # the other half of k~/v~ arrives via a pair AllGather.
    wkvhp = nc.dram_tensor("wkvhp", [NI, P, H], R, kind="ExternalInput").ap()
    widkvhp = nc.dram_tensor("widkvhp", [NI, P, H], R, kind="ExternalInput").ap()
    woutT = nc.dram_tensor("woutT", [H, H], R, kind="ExternalInput").ap()
    boutb = nc.dram_tensor("boutb", [P, H], F32, kind="ExternalInput").ap()
    out = nc.dram_tensor("out", [SC, H], F32, kind="ExternalOutput").ap()

    with tile.TileContext(nc) as tc:
        with tc.tile_pool(name="pers", bufs=1) as pers:
            qTp_cm = tc.tile_pool(name="qTp", bufs=1)
            qTp = qTp_cm.__enter__()
            # ---- persistent constants / arrays --------------------------------
            ones_mat = pers.tile([P, P], R, tag="ones_mat")
            nc.vector.memset(ones_mat[:, :], 1.0)
            bias_col = pers.tile([P, 1], F32, tag="bias_col")
            # engine ops need 32-aligned start partitions: write the gap
            # as [64:96] then restore [64:77]; later writes overwrite cleanly.
            nc.vector.memset(bias_col[:, :], 0.0)
            nc.vector.memset(bias_col[64:GAP1, :], -1e30)
            nc.vector.memset(bias_col[64:GAP0, :], 0.0)
            kT_sb = [pers.tile([P, LP], R, tag=f"kT{j}", name=f"kT{j}") for j in range(NJ)]
            v_sb = pers.tile([LP, HEADS * DH], R, tag="v")

            qT_sb = [qTp.tile([P, SC], R, tag=f"qT{j}", name=f"qT{j}") for j in range(NJ)]

            # ---- phase Q: q projection + k/v half-projection + pair exchange --
            with (
                tc.tile_pool(name="phq", bufs=1) as phq,
                tc.tile_pool(name="wqs", bufs=4) as wqs,
                tc.tile_pool(name="wkvs", bufs=2) as wkvs,
                tc.tile_pool(name="ccdram", bufs=1, space="DRAM") as ccdram,
            ):
                psq_cm = tc.tile_pool(name="psq", bufs=4, space="PSUM")
                psq = psq_cm.__enter__()
                pskv_cm = tc.tile_pool(name="pskv", bufs=3, space="PSUM")
                pskv = pskv_cm.__enter__()
                hsT_sb = [phq.tile([P, SC], R, tag=f"hsT{i}", name=f"hsT{i}") for i in range(NJ)]
                for i in range(NJ):
                    nc.sync.dma_start(out=hsT_sb[i][:, :], in_=hsT[ts(i, P), :])
                xkvT_sb = [phq.tile([P, LP], R, tag=f"xkvT{i}", name=f"xkvT{i}") for i in range(NI)]
                for i in range(NI):
                    nc.sync.dma_start(out=xkvT_sb[i][:, :], in_=xkvTp[i])
                kTMP = phq.tile([P, H], R, tag="kTMP")
                kvloc = phq.tile([P, H], R, tag="kvloc")
                cc_in = ccdram.tile([P, H], R, tag="cc_in")
                cc_out = ccdram.tile([2, P, H], R, tag="cc_out")

                def q_group(j):
                    pss = [psq.tile([P, 512], F32, tag="qps", name="qps") for _ in range(NCH)]
                    for i in range(NJ):
                        wq_t = wqs.tile([P, P], R, tag="wq", name="wq_t")
                        nc.sync.dma_start(out=wq_t[:, :], in_=wqp[j, i])
                        for c in range(NCH):
                            nc.tensor.matmul(
                                pss[c][:, :], wq_t[:, :], hsT_sb[i][:, ts(c, 512)],
                                start=(i == 0), stop=(i == NJ - 1),
                            )
                    for c in range(NCH):
                        nc.scalar.copy(qT_sb[j][:, ts(c, 512)], pss[c][:, :])

                # kv half-projection: src 0 = [Wk|Wv] half (valid rows 0:77 +
                # zero gap), src 1 = [Wid_k|Wid_v] half (valid rows 96:128).
                # Full [128, x] psum is DMA'd straight into cc_in; src 1
                # overwrites rows 96:128 afterwards.
                kv_state = {}

                def kv_step(srcidx, i):
                    w_src = wkvhp if srcidx == 0 else widkvhp
                    w_t = wkvs.tile([P, H], R, tag="wkv", name="wkv_t")
                    nc.sync.dma_start(out=w_t[:, :], in_=w_src[i])
                    if i == 0:
                        kv_state[srcidx] = [
                            pskv.tile([P, mw], F32, tag="kvps", name="kvps")
                            for (m0, mw) in MCHUNKS
                        ]
                    pss = kv_state[srcidx]
                    for mi, (m0, mw) in enumerate(MCHUNKS):
                        nc.tensor.matmul(
                            pss[mi][:, :], xkvT_sb[i][:, :], w_t[:, m0:m0 + mw],
                            start=(i == 0), stop=(i == NI - 1),
                        )
                    if i == NI - 1:
                        for mi, (m0, mw) in enumerate(MCHUNKS):
                            if srcidx == 0:
                                nc.scalar.copy(kvloc[:, m0:m0 + mw], pss[mi][:, :])
                            else:
                                nc.scalar.copy(
                                    kvloc[GAP1:P, m0:m0 + mw], pss[mi][GAP1:P, :])
                        if srcidx == 1:
                            nc.sync.dma_start(out=cc_in[:, :], in_=kvloc[:, :])

                kv_steps = [(srcidx, i) for srcidx in (0, 1) for i in range(NI)]
                ki = 0
                for j in range(NJ):
                    q_group(j)
                    for _ in range(7):
                        if ki < len(kv_steps):
                            kv_step(*kv_steps[ki])
                            ki += 1

                pskv_cm.__exit__(None, None, None)
                psq_cm.__exit__(None, None, None)

                # pair exchange: core 2b holds cols 0:640, core 2b+1 cols 640:1280
                nc.gpsimd.collective_compute(
                    "AllGather", mybir.AluOpType.bypass,
                    ins=[cc_in[:, :]], outs=[cc_out[:, :, :]],
                    replica_groups=[[0, 1], [2, 3], [4, 5], [6, 7]],
                )
                HH = H // 2
                nc.sync.dma_start(out=v_sb[:, 0:HH], in_=cc_out[0, :, HH:H])
                nc.sync.dma_start(out=v_sb[:, HH:H], in_=cc_out[1, :, HH:H])
                # k: reload per 128-col chunk and DMA-transpose immediately so
                # the chunks pipeline under the remaining q-proj groups.
                for j in range(NJ):
                    half, off = (0, 0) if j < NJ // 2 else (1, HH)
                    nc.sync.dma_start(out=kTMP[:, ts(j, P)],
                                      in_=cc_out[half, :, P * j - off:P * (j + 1) - off])
                    nc.sync.dma_start(out=kT_sb[j][:, :], in_=kTMP[:, ts(j, P)],
                                      transpose=True)

            # ---- phase A: attention -------------------------------------------
            attnp_cm = tc.tile_pool(name="attnp", bufs=1, side="right")
            attnp = attnp_cm.__enter__()
            attnT_sb = [attnp.tile([P, SC], R, tag=f"attnT{d}", name=f"attnT{d}") for d in range(NJ)]
            boutb_sb = attnp.tile([P, H], F32, tag="boutb")
            nc.sync.dma_start(out=boutb_sb[:, :], in_=boutb)
            wout_sb = [attnp.tile([P, H], R, tag=f"wout{i}", name=f"wout{i}") for i in range(NJ)]
            for i in range(NJ):
                nc.sync.dma_start(out=wout_sb[i][:, :], in_=woutT[ts(i, P), :])
            with (
                tc.tile_pool(name="pha", bufs=3) as pha,
                tc.tile_pool(name="psa", bufs=2, space="PSUM") as psa,
            ):
                # software pipeline: scores+exp of item idx run while
                # PV/denominator/normalize of item idx-1 occupy the PE/DVE, so
                # the PE never waits on the ACT exp latency.
                items = [(c, hp, s) for c in range(NCH) for hp in range(NJ) for s in range(2)]
                astate = {}

                def attn_front(idx):
                    c, hp, s = items[idx]
                    rq = DH * s
                    ps_s = psa.tile([P, 512], F32, tag="sps", name="sps")
                    nc.tensor.matmul(
                        ps_s[:, :], kT_sb[hp][rq:rq + DH, :],
                        qT_sb[hp][rq:rq + DH, ts(c, 512)],
                        start=True, stop=True,
                    )
                    probsT = pha.tile([P, 512], R, tag="probsT", name="probsT")
                    nc.scalar.activation(
                        probsT[:, :], ps_s[:, :], EXP,
                        bias=bias_col[:, :], scale=SCALE,
                    )
                    astate[idx] = probsT

                def attn_back(idx):
                    c, hp, s = items[idx]
                    rq = DH * s
                    probsT = astate.pop(idx)
                    # v for BOTH heads of the pair as stationary (fp32r needs
                    # M=128); only rows rq:rq+64 belong to this head.
                    ps_o = psa.tile([P, 512], F32, tag="ops", name="ops")
                    nc.tensor.matmul(
                        ps_o[:, :], v_sb[:, ts(hp, P)], probsT[:, :],
                        start=True, stop=True,
                    )
                    # all-ones stationary -> denominator replicated on every
                    # partition; reciprocal then multiplies the valid rows.
                    ps_d = psa.tile([P, 512], F32, tag="dps", name="dps")
                    nc.tensor.matmul(
                        ps_d[:, :], ones_mat[:, :], probsT[:, :],
                        start=True, stop=True,
                    )
                    bc_sb = pha.tile([P, 512], F32, tag="bc", name="bc_sb")
                    nc.vector.reciprocal_approx_fast(bc_sb[:, :], ps_d[:, :])
                    nc.vector.tensor_mul(
                        attnT_sb[hp][rq:rq + DH, ts(c, 512)],
                        ps_o[rq:rq + DH, :], bc_sb[rq:rq + DH, :]
                    )

                for idx in range(len(items)):
                    attn_front(idx)
                    if idx >= 1:
                        attn_back(idx - 1)
                attn_back(len(items) - 1)

            # release qT before phase O (attnp on the right stack stays open)
            qTp_cm.__exit__(None, None, None)

            # ---- phase O: output projection + bias ----------------------------
            with (
                tc.tile_pool(name="finp", bufs=3) as finp,
                tc.tile_pool(name="pso", bufs=4, space="PSUM") as pso,
            ):
                for t in range(NT):
                    fin = finp.tile([P, H], F32, tag="fin", name="fin")
                    for m0, mw in MCHUNKS:
                        psf = pso.tile([P, mw], F32, tag="psf", name="psf")
                        for i in range(NJ):
                            nc.tensor.matmul(
                                psf[:, :], attnT_sb[i][:, ts(t, P)],
                                wout_sb[i][:, m0:m0 + mw],
                                start=(i == 0), stop=(i == NJ - 1),
                            )
                        nc.vector.tensor_add(
                            fin[:, m0:m0 + mw], psf[:, :], boutb_sb[:, m0:m0 + mw]
                        )
                    nc.sync.dma_start(out=out[ts(t, P), :], in_=fin[:, :])

            attnp_cm.__exit__(None, None, None)

    nc.compile()
    _NC_CACHE["nc"] = nc
    return nc


def prep_core_inputs(hidden_states, encoder_hidden_states, id_embedding,
                     Wq, Wk, Wv, Wid_k, Wid_v, Wout, bout):
    """Host-side sharding / layout prep.  Returns list of 8 in_maps."""
    f = np.float32
    h16 = np.float16
    hidden_states = np.asarray(hidden_states, f)
    encoder_hidden_states = np.asarray(encoder_hidden_states, f)
    id_embedding = np.asarray(id_embedding, f)
    Wq = np.asarray(Wq, f)
    Wout = np.asarray(Wout, f)
    Wk, Wv = np.asarray(Wk, f), np.asarray(Wv, f)
    Wid_k, Wid_v = np.asarray(Wid_k, f), np.asarray(Wid_v, f)
    boutb = np.ascontiguousarray(np.broadcast_to(np.asarray(bout, f), (P, H)))

    # packed tile-major weight layouts (contiguous DMA tiles)
    wqp = np.ascontiguousarray(
        Wq.reshape(NJ, P, NJ, P).transpose(2, 0, 1, 3).astype(h16))            # [j,i,128,128]
    # per-parity halves of the kv weights: core parity hf computes k~/v~
    # columns [640*hf, 640*(hf+1)); the pair AllGather restores the full k/v.
    HH = H // 2
    wkvh, widkvh = [], []
    for hf in range(2):
        cols = slice(HH * hf, HH * (hf + 1))
        wk = np.concatenate([Wk[:, cols], Wv[:, cols]], axis=1)                # [C, H]
        wi = np.concatenate([Wid_k[:, cols], Wid_v[:, cols]], axis=1)
        wkvh.append(np.ascontiguousarray(wk.reshape(NI, P, H).astype(h16)))
        widkvh.append(np.ascontiguousarray(wi.reshape(NI, P, H).astype(h16)))

    in_maps = []
    for core in range(NCORES):
        b, hf = divmod(core, 2)
        hsT = np.ascontiguousarray(hidden_states[b, hf * SC:(hf + 1) * SC, :].T.astype(h16))
        xkvT = np.zeros((C, LP), h16)                                          # [C, 128]
        xkvT[:, :TE] = encoder_hidden_states[b].T
        xkvT[:, GAP1:] = id_embedding[b % 2].T
        xkvTp = np.ascontiguousarray(xkvT.reshape(NI, P, LP))
        in_maps.append({
            "hsT": hsT, "xkvTp": xkvTp, "wqp": wqp, "wkvhp": wkvh[hf],
            "widkvhp": widkvh[hf], "woutT": np.ascontiguousarray(Wout.astype(h16)), "boutb": boutb,
        })
    return in_maps


def kernel(hidden_states, encoder_hidden_states, id_embedding,
           Wq, Wk, Wv, Wid_k, Wid_v, Wout, bout, _trace=False):
    _ensure_axon_hooks()
    from concourse.bass_utils import run_bass_kernel_spmd

    nc = build_nc()
    in_maps = prep_core_inputs(hidden_states, encoder_hidden_states, id_embedding,
                               Wq, Wk, Wv, Wid_k, Wid_v, Wout, bout)
    kwargs = {}
    if _trace:
        import concourse.bass_utils as bu
        bu.upload_artifacts = lambda tmpdir: f"local://{tmpdir}"
        kwargs["trace"] = True
    res = run_bass_kernel_spmd(nc, in_maps, core_ids=list(range(NCORES)), **kwargs)

    outp = np.empty((B, S, H), np.float32)
    for core in range(NCORES):
        b, hf = divmod(core, 2)
        outp[b, hf * SC:(hf + 1) * SC, :] = res.results[core]["out"]
    if _trace:
        kernel.last_exec_time_ns = res.exec_time_ns
        kernel.last_results = res
    return outp
